# revision 13
# baseline (speedup 1.0000x reference)
"""DDNLoss (depth distribution network focal loss) on 8 trn2 NeuronCores.

Data-parallel over B (1 image per core, B=8).

Key observation: the focal loss per pixel depends only on
S = sum_c exp(logit_c) and on logit_target — the order of channels
within a pixel's 81-way softmax is irrelevant. The host therefore
prepares, per image (cheap numpy on the 256-box control plane + one
relayout pass over the logits):
  * the LID target-bin map t(h,w) and fg/bg weight map (tiny),
  * pixel-major logits (HW, 81) in bf16 with each pixel's target
    channel swapped into slot 0, laid out as (128, 234*81) so each
    partition holds 234 consecutive pixels contiguously.

The device then does all the floating-point math:
  * exp over all 81*29952 logits per core (ACT, 128 lanes, 13 chunks),
  * per-pixel 81-element sums via one 3D-AP tensor_reduce per chunk
    (DVE, (128, 18, 81) -> (128, 18)),
  * e_t = slot-0 strided view of exp(logits),
  * focal epilogue on (128, 234): p = e_t/S, -alpha*(1-p)^2*ln(p)*w,
    free-dim accumulate -> per-partition partials (128, 1).
Host sums the 8x128 partials (the "all-reduce") -> scalar loss.

All ACT functions used (Exp, Ln, Square) live in the
natural_log_exp_and_others table set; a doctored table list plus a
dummy warm-up Exp forces a single ACT_TABLE_LOAD in the preamble
instead of two loads on the critical path.
"""

import numpy as np
import ml_dtypes
from contextlib import ExitStack

import concourse.bass as bass
import concourse.bacc as bacc_mod
import concourse.tile as tile
import concourse.mybir as mybir
from concourse.bass_utils import run_bass_kernel_spmd
from concourse.hw_specs import get_activation_tables

# Problem constants (hardcoded per contract)
B, C, H, W, N = 8, 81, 96, 312, 32
HW = H * W                      # 29952
P = 128
PPP = HW // P                   # 234 pixels per partition (block layout)
FREE = PPP * C                  # 18954 elements per partition
CP = 84                         # padded channels per pixel on-chip (3 zero pads)
CH = CP // 2                    # 42: pair-sum halves
FREEP = PPP * CP                # 19656 padded elements per partition
# chunk sizes in pixel-groups: small head chunks start the pipeline early
CHUNKS = [2, 4, 12] + [18] * 11 + [12, 6]
assert sum(CHUNKS) == PPP

ALPHA = 0.25
D_MIN, D_MAX, NUM_BINS = 0.001, 60.0, 80
BIN_SIZE = 2.0 * (D_MAX - D_MIN) / (NUM_BINS * (1 + NUM_BINS))
FG_W, BG_W = 13.0, 1.0
C0 = -ALPHA / float(B * HW)     # fold -alpha and global pixel normalizer

ACT_SET = "natural_log_exp_and_others"

LAST_RESULTS = None


class _Bacc(bacc_mod.Bacc):
    """Bacc whose activation-table chooser only sees ACT_SET, so every
    activation resolves to that one set (1 table load total). Set order
    and indices are preserved; only the membership info the chooser uses
    is masked."""

    def insert_act_table_loads(self):
        has_activation = any(
            isinstance(i, mybir.InstActivation)
            for b in self.main_func.blocks
            for i in b.instructions
        )
        if not has_activation:
            return
        tables = [
            (name, (fns if name == ACT_SET else set()))
            for name, fns in get_activation_tables(self.m.arch).items()
        ]
        import bass_rust as _bass_rust
        _bass_rust.insert_act_table_loads(self, tables)


def build_program():
    f32 = mybir.dt.float32
    bf16 = mybir.dt.bfloat16
    Alu = mybir.AluOpType
    Act = mybir.ActivationFunctionType

    nc = _Bacc("TRN2", target_bir_lowering=False)
    lrot = nc.dram_tensor("lrot", [P, FREE], bf16, kind="ExternalInput")
    wc0 = nc.dram_tensor("wc0", [P, PPP], f32, kind="ExternalInput")
    partial = nc.dram_tensor("partial", [P, 1], f32, kind="ExternalOutput")

    with ExitStack() as ctx:
        tc = ctx.enter_context(tile.TileContext(nc))
        consts = ctx.enter_context(tc.tile_pool(name="consts", bufs=1))
        lg = ctx.enter_context(tc.tile_pool(name="lg", bufs=6))
        fin = ctx.enter_context(tc.tile_pool(name="fin", bufs=1))

        zero128 = consts.tile([P, 1], f32)
        nc.vector.memset(zero128[:], 0.0)
        nc.const_aps.aps[(f32, 0.0)] = zero128[:]

        # Warm-up: 1-element Exp so the single ACT table load lands in
        # the preamble, overlapping the first chunk DMA.
        warm = consts.tile([1, 1], f32)
        nc.scalar.activation(warm[:], zero128[0:1], Act.Exp)

        # exp(logits) in 84-stride pixel rows (slot 0 = target, 81..83 pad)
        ebig = fin.tile([P, FREEP], bf16)
        h1 = fin.tile([P, PPP * CH], bf16)  # pair-sums (42 per pixel)
        sbig = fin.tile([P, PPP], f32)      # per-pixel sum of exp
        wt = fin.tile([P, PPP], f32)

        # zero the 3 pad columns of every pixel row once, so the pair-sum
        # can read the full 84-wide rows
        nc.vector.memset(
            ebig[:].rearrange("p (s n) -> p s n", n=CP)[:, :, C:CP], 0.0)

        g0 = 0
        for ng in CHUNKS:
            gsl = slice(g0, g0 + ng)
            sl = slice(g0 * C, (g0 + ng) * C)           # dense (DRAM) extent
            Lj = lg.tile([P, ng * C], bf16, tag=f"L{ng}")
            nc.sync.dma_start(Lj[:], lrot[:, sl])
            ev = ebig[:].rearrange("p (s n) -> p s n", n=CP)[:, gsl, 0:C]
            nc.scalar.activation(ev, Lj[:].rearrange("p (s n) -> p s n", n=C),
                                 Act.Exp)
            e3 = ebig[:].rearrange("p (s n) -> p s n", n=CP)[:, gsl, :]
            h3 = h1[:].rearrange("p (s n) -> p s n", n=CH)[:, gsl, :]
            nc.vector.tensor_tensor(
                out=h3, in0=e3[:, :, 0:CH], in1=e3[:, :, CH:CP], op=Alu.add)
            nc.vector.tensor_reduce(
                out=sbig[:][:, gsl], in_=h3,
                axis=mybir.AxisListType.X, op=Alu.add)
            g0 += ng

        nc.sync.dma_start(wt[:], wc0[:, :])

        # ---- focal epilogue on (128, 234)
        rs = fin.tile([P, PPP], f32)
        nc.vector.reciprocal_approx_fast(rs[:], sbig[:])
        et = ebig[:][:, 0:FREEP:CP]         # (128, 234) bf16 strided
        p_ = fin.tile([P, PPP], f32)
        nc.vector.tensor_tensor(out=p_[:], in0=et, in1=rs[:], op=Alu.mult)
        logp = fin.tile([P, PPP], f32)
        nc.scalar.activation(logp[:], p_[:], Act.Ln)
        om2 = fin.tile([P, PPP], f32)       # (1-p)^2 via free affine+Square
        nc.scalar.activation(om2[:], p_[:], Act.Square, bias=1.0, scale=-1.0)
        t2 = fin.tile([P, PPP], f32)
        nc.vector.tensor_tensor(out=t2[:], in0=om2[:], in1=logp[:],
                                op=Alu.mult)
        fs = fin.tile([P, PPP], f32)
        acc = fin.tile([P, 1], f32)
        nc.vector.scalar_tensor_tensor(
            out=fs[:], in0=t2[:], scalar=0.0, in1=wt[:],
            op0=Alu.add, op1=Alu.mult, accum_out=acc[:])
        nc.sync.dma_start(partial[:, :], acc[:])

    nc.compile()
    return nc


_CACHE = {}


def _get_program():
    if "nc" not in _CACHE:
        _CACHE["nc"] = build_program()
    return _CACHE["nc"]


def _host_maps(boxes, depth):
    """Per-image target-bin map t (H,W) int32 and weight map (H,W) f32,
    mirroring the reference's float32 math."""
    u1 = np.floor(boxes[:, 0]).astype(np.int32)
    v1 = np.floor(boxes[:, 1]).astype(np.int32)
    u2 = np.ceil(boxes[:, 2]).astype(np.int32)
    v2 = np.ceil(boxes[:, 3]).astype(np.int32)
    rows = np.arange(H, dtype=np.int32)
    cols = np.arange(W, dtype=np.int32)
    mv = (rows[None, :] >= v1[:, None]) & (rows[None, :] < v2[:, None])  # (N,H)
    mu = (cols[None, :] >= u1[:, None]) & (cols[None, :] < u2[:, None])  # (N,W)
    mask = mv[:, :, None] & mu[:, None, :]                               # (N,H,W)
    cand = np.where(mask, depth[:, None, None].astype(np.float32), np.inf)
    dm = cand.min(axis=0)
    fg = mask.any(axis=0)
    dm = np.where(fg, dm, np.float32(0.0)).astype(np.float32)
    idx = np.float32(-0.5) + np.float32(0.5) * np.sqrt(
        np.float32(1.0) + np.float32(8.0 / BIN_SIZE) * (dm - np.float32(D_MIN)))
    invalid = (idx < 0) | (idx > NUM_BINS) | ~np.isfinite(idx)
    t = np.where(invalid, NUM_BINS, idx).astype(np.int32)                # (H,W)
    w = np.where(fg, np.float32(FG_W), np.float32(BG_W)) * np.float32(C0)
    return t, w.astype(np.float32)


def kernel(depth_logits, gt_boxes2d, num_gt_per_img, gt_center_depth):
    global LAST_RESULTS
    dl = np.ascontiguousarray(np.asarray(depth_logits, dtype=np.float32))
    assert dl.shape == (B, C, H, W), dl.shape
    n_gt = int(num_gt_per_img)
    assert n_gt == N, n_gt
    boxes = np.asarray(gt_boxes2d, dtype=np.float32)
    depth = np.asarray(gt_center_depth, dtype=np.float32)

    pidx = np.arange(HW)
    in_maps = []
    for b in range(B):
        sl = slice(b * N, (b + 1) * N)
        t, w = _host_maps(boxes[sl], depth[sl])
        # pixel-major logits with target channel swapped into slot 0
        LT = np.ascontiguousarray(dl[b].reshape(C, HW).T)  # (HW, 81)
        tf = t.reshape(HW)
        l0 = LT[pidx, 0].copy()
        LT[pidx, 0] = LT[pidx, tf]
        LT[pidx, tf] = l0
        lrot = LT.astype(ml_dtypes.bfloat16).reshape(P, FREE)
        in_maps.append({
            "lrot": np.ascontiguousarray(lrot),
            "wc0": np.ascontiguousarray(w.reshape(P, PPP)),
        })

    nc = _get_program()
    res = run_bass_kernel_spmd(nc, in_maps, core_ids=list(range(B)))
    LAST_RESULTS = res
    total = np.float64(0.0)
    for r in res.results:
        total += np.asarray(r["partial"], dtype=np.float64).sum()
    return np.float32(total)


if __name__ == "__main__":
    import tempfile
    from concourse.bass_utils import compile_bass_kernel
    compile_bass_kernel(_get_program(), tempfile.mkdtemp())
    print("COMPILE OK")
